# revision 16
# baseline (speedup 1.0000x reference)
"""Trainium2 Bass kernel for nn_CP_Attention_Action (dense transformer block with
CP-factored low-rank corrections).

Data-parallel over batch B=8 -> one batch per NeuronCore, no collectives.
Per core, feature-on-partition (transposed) layout:
  qkT (o,n) via stationary W tiles (CP branch fused into the PSUM group);
  v in natural (m,d) layout so it is the attn@v stationary operand, with a
  ones column appended so the softmax denominator falls out of the same
  matmul; logits via 64x128 row-tiled matmul pairs (two heads concurrently);
  exp on ScalarE with fused mask-bias + scale; 1/denom via K=1 bf16 matmul
  broadcast of the raw denominator + reciprocal_approx_fast on DVE; proj
  (+CP branch) in transposed layout; host transposes (o,n) -> (n,o).
"""

import numpy as np
import ml_dtypes

import concourse.bass as bass
from concourse import bacc
import concourse.mybir as mybir
import concourse.tile as tile
from concourse.bass_utils import run_bass_kernel_spmd

B, N, DIM = 8, 1024, 512
H, D = 8, 64
R = 64
SCALE = D ** -0.5
NCORES = 8
NC_CH = 2          # n chunks of 512
NT = N // 128      # 8 m-tiles
CT = DIM // 128    # 4 c-tiles
OT = 8             # q,k o-tiles
F32 = mybir.dt.float32
BF = mybir.dt.bfloat16
AF = mybir.ActivationFunctionType
bf16 = ml_dtypes.bfloat16

_CACHE = {}


def _build():
    nc = bacc.Bacc()

    xT_e = nc.declare_dram_parameter("xT", [DIM, N], BF, isOutput=False)
    wqkvT_e = nc.declare_dram_parameter("wqkvT", [DIM, 3 * DIM], BF, isOutput=False)
    cpuwT_e = nc.declare_dram_parameter("cpuwT", [DIM, R], BF, isOutput=False)
    cpvwT_e = nc.declare_dram_parameter("cpvwT", [R, DIM], BF, isOutput=False)
    cpvw65_e = nc.declare_dram_parameter("cpvw65", [R + 1, DIM], BF, isOutput=False)
    cpcrt_e = nc.declare_dram_parameter("cpcrt", [R, R * R], BF, isOutput=False)
    cpatt_e = nc.declare_dram_parameter("cpatt", [R, 4], BF, isOutput=False)
    wprojT_e = nc.declare_dram_parameter("wprojT", [DIM, DIM], BF, isOutput=False)
    fcon_e = nc.declare_dram_parameter("fcon", [128, 32], F32, isOutput=False)
    out_e = nc.declare_dram_parameter("out", [DIM, N], F32, isOutput=True)

    fdram = nc.dram_tensor("fdram", [4, R * R], BF)

    # fcon layout (f32 columns): 0:4 cpvb | 4:8 pbias | 8:16 maskb | 16 cpub(rows 0:64)
    with tile.TileContext(nc) as tc:
        with tc.tile_pool(name="consts", bufs=1) as consts, \
             tc.tile_pool(name="qkpool", bufs=1) as qkpool, \
             tc.tile_pool(name="stpool", bufs=3) as stpool, \
             tc.tile_pool(name="work", bufs=1) as work, \
             tc.tile_pool(name="dnpool", bufs=3) as dnpool, \
             tc.tile_pool(name="outp", bufs=1) as outp, \
             tc.tile_pool(name="popool", bufs=2) as popool:

            # ---------- constants / inputs (F-path + x first, weights split) ----------
            cbf = consts.tile([128, 1536], BF)
            nc.sync.dma_start(cbf[0:R, 1024:1028], cpatt_e[:, :])
            cpcrt = consts.tile([R, R * R], BF)
            nc.sync.dma_start(cpcrt[:], cpcrt_e[:, :])
            xT = consts.tile([128, CT, N], BF)
            nc.sync.dma_start(xT[:], xT_e[:, :].rearrange("(ct p) n -> p ct n", p=128))
            cpuw = consts.tile([128, CT, R], BF)
            nc.sync.dma_start(cpuw[:], cpuwT_e[:, :].rearrange("(ct p) r -> p ct r", p=128))
            fcon = consts.tile([128, 32], F32)
            nc.sync.dma_start(fcon[:], fcon_e[:, :])
            wqkv = consts.tile([128, CT, 3 * DIM], BF)
            nc.gpsimd.dma_start(wqkv[:], wqkvT_e[:, :].rearrange("(ct p) o -> p ct o", p=128))
            nc.gpsimd.dma_start(cbf[0:R + 1, 0:DIM], cpvw65_e[:, :])
            nc.gpsimd.dma_start(cbf[0:R, DIM:2 * DIM], cpvwT_e[:, :])
            wproj = consts.tile([128, CT, DIM], BF)
            nc.gpsimd.dma_start(wproj[:], wprojT_e[:, :].rearrange("(ct p) o -> p ct o", p=128))
            ones_bf = consts.tile([1, R], BF)
            nc.vector.memset(ones_bf[:], 1.0)

            def cpvw65(): return cbf[0:R + 1, 0:DIM]
            def cpvw(dt_): return cbf[0:R, DIM + dt_ * 128:DIM + (dt_ + 1) * 128]
            def cpatt(): return cbf[0:R, 1024:1028]
            def F_sb(f): return cbf[0:R, 1028 + f * R:1028 + (f + 1) * R]
            def cpvb(dt_): return fcon[:, dt_:dt_ + 1]
            def pbias(dt_): return fcon[:, 4 + dt_:5 + dt_]
            def maskb(mt): return fcon[:, 8 + mt:9 + mt]
            def cpub(): return fcon[0:R, 16:17]

            # ---------- F = CP_C x CP_attention (transient pools) ----------
            with tc.tile_pool(name="fsb", bufs=1) as fsb, \
                 tc.tile_pool(name="psf", bufs=1, space="PSUM") as psf:
                fp = psf.tile([4, R * R], F32)
                for ch in range(8):
                    nc.tensor.matmul(fp[:, ch * 512:(ch + 1) * 512],
                                     lhsT=cpatt(), rhs=cpcrt[0:R, ch * 512:(ch + 1) * 512],
                                     start=True, stop=True)
                fflat = fsb.tile([4, R * R], BF)
                nc.vector.tensor_copy(fflat[:], fp[:])
                nc.sync.dma_start(fdram[:, :], fflat[0:4, :])
                for f in range(4):
                    nc.sync.dma_start(cbf[0:R, 1028 + f * R:1028 + (f + 1) * R],
                                      fdram[f].rearrange("(r s) -> r s", s=R))

            outT = outp.tile([128, CT, N], BF)
            uu = work.tile([128, 2, N], BF)   # rows 0:64 -> [0]=u, [1]=u2
            tt = work.tile([128, 3, N], BF)   # rows 0:65; [2] has ones row for v
            v_sb = work.tile([128, NT, H, D + 1], BF)
            qk = qkpool.tile([128, OT, N], BF)

            # ================= phase 1: u/t, v, first qk pair =================
            with tc.tile_pool(name="ps_pre", bufs=3, space="PSUM") as ps_pre, \
                 tc.tile_pool(name="ps_sm", bufs=2, space="PSUM") as ps_sm:

                # u = CP_U(x)
                ups = []
                for ncx in range(NC_CH):
                    nsl = slice(ncx * 512, (ncx + 1) * 512)
                    up = ps_sm.tile([R, 512], F32, tag="sm", name=f"up{ncx}")
                    for ct in range(CT):
                        nc.tensor.matmul(up[:, :], lhsT=cpuw[:, ct, :], rhs=xT[:, ct, nsl],
                                         start=(ct == 0), stop=(ct == CT - 1))
                    nc.vector.tensor_scalar_add(uu[0:R, 0, nsl], up[:, :], cpub())

                # qk main matmuls for the first pair (no t dependency yet)
                qk04 = {}
                for ot in (0, 4):
                    qps = [ps_pre.tile([128, 512], F32, tag="big", name=f"qp{ot}_{i}")
                           for i in range(NC_CH)]
                    qk04[ot] = qps
                    for ct in range(CT):
                        for ncx in range(NC_CH):
                            nc.tensor.matmul(qps[ncx][:, :],
                                             lhsT=wqkv[:, ct, ot * 128:(ot + 1) * 128],
                                             rhs=xT[:, ct, ncx * 512:(ncx + 1) * 512],
                                             start=(ct == 0), stop=False)

                # t_f = F_f.T @ uT
                for f in range(3):
                    for ncx in range(NC_CH):
                        nsl = slice(ncx * 512, (ncx + 1) * 512)
                        tp = ps_sm.tile([R, 512], F32, tag="sm", name=f"tp{f}_{ncx}")
                        nc.tensor.matmul(tp[:, :], lhsT=F_sb(f), rhs=uu[0:R, 0, nsl],
                                         start=True, stop=True)
                        nc.vector.tensor_copy(tt[0:R, f, nsl], tp[:, :])
                nc.vector.memset(tt[R:R + 1, 2, :], 1.0)

                # finish qk 0/4: CP add + evac
                for ot in (0, 4):
                    f = 0 if ot < 4 else 1
                    dt_ = ot % 4
                    for ncx in range(NC_CH):
                        nsl = slice(ncx * 512, (ncx + 1) * 512)
                        nc.tensor.matmul(qk04[ot][ncx][:, :], lhsT=cpvw(dt_),
                                         rhs=tt[0:R, f, nsl], start=False, stop=True)
                        nc.vector.tensor_scalar_add(qk[:, ot, nsl], qk04[ot][ncx][:, :],
                                                    cpvb(dt_))
                nc.vector.memset(v_sb[:, :, :, D:D + 1], 1.0)

            # ====== phase 2: attention ======
            # Fine-grained interleave: while the logits/exp stream of pair p
            # runs (exp on ScalarE is the pair-rate limiter), the PE executes
            # attn@v matmuls of pair p-1 (and, for pair 0, the v/qk
            # production) between logits tiles, so neither engine starves.
            # The denominator broadcast is col-tiled into partitions 64:128
            # of the attn@v psum tile (no extra bank).
            with tc.tile_pool(name="ps_log", bufs=2, space="PSUM") as ps_log, \
                 tc.tile_pool(name="ps_av", bufs=2, space="PSUM") as ps_av, \
                 tc.tile_pool(name="ps_bc", bufs=2, space="PSUM") as ps_bc:

                def denom_chain(ap_, hl, pair, ncx):
                    nsl = slice(ncx * 512, (ncx + 1) * 512)
                    den = dnpool.tile([1, 512], BF, tag="den", name=f"den{pair}_{hl}_{ncx}")
                    nc.vector.tensor_copy(den[0:1, :], ap_[D:D + 1, :])
                    bcp = ps_bc.tile([D, 512], F32, tag="bc", name=f"bcp{pair}_{hl}_{ncx}")
                    nc.tensor.matmul(bcp[0:D, :], lhsT=ones_bf[0:1, 0:D],
                                     rhs=den[0:1, :], start=True, stop=True)
                    bc = dnpool.tile([D, 512], F32, tag="bcs", name=f"bc{pair}_{hl}_{ncx}")
                    nc.vector.reciprocal_approx_fast(bc[0:D, :], bcp[0:D, :])
                    nc.vector.tensor_mul(outT[hl * 64:hl * 64 + 64, pair, nsl],
                                         ap_[0:D, :], bc[0:D, :])

                def av_tile(box, key, name):
                    if key not in box:
                        box[key] = ps_av.tile([128, 512], F32, tag="av", name=name)
                    return box[key]

                def make_av_filler(pair, st_pair):
                    # 32 attn@v MMs + 4 denom chains for `pair`, lazy-allocated
                    items = []
                    box = {}
                    for hl in range(2):
                        h = 2 * pair + hl
                        for mt in range(NT):
                            for ncx in range(NC_CH):
                                def mm(hl=hl, mt=mt, ncx=ncx, h=h):
                                    ap_ = av_tile(box, (hl, ncx), f"ap{pair}_{hl}_{ncx}")
                                    nc.tensor.matmul(
                                        ap_[0:D + 1, :], lhsT=v_sb[:, mt, h, :],
                                        rhs=st_pair[hl][:, mt, ncx * 512:(ncx + 1) * 512],
                                        start=(mt == 0), stop=(mt == NT - 1))
                                items.append(mm)
                        for ncx in range(NC_CH):
                            def dn(hl=hl, ncx=ncx):
                                denom_chain(box[(hl, ncx)], hl, pair, ncx)
                            items.append(dn)
                    return items

                def make_p1_filler():
                    # v (m, d) groups + remaining qk tiles under pair-0's stream
                    items = []
                    for mt in range(NT):
                        box = {}
                        for ct in range(CT):
                            def mm(box=box, ct=ct, mt=mt):
                                vp = av_tile(box, "vp", f"vp{mt}")
                                nc.tensor.matmul(
                                    vp[:, :], lhsT=xT[:, ct, mt * 128:(mt + 1) * 128],
                                    rhs=wqkv[:, ct, 2 * DIM:3 * DIM],
                                    start=(ct == 0), stop=False)
                            items.append(mm)
                        def mm2(box=box, mt=mt):
                            nc.tensor.matmul(
                                box["vp"][:, :],
                                lhsT=tt[0:R + 1, 2, mt * 128:(mt + 1) * 128],
                                rhs=cpvw65(), start=False, stop=True)
                        def ev(box=box, mt=mt):
                            nc.vector.tensor_copy(
                                v_sb[:, mt, :, 0:D],
                                box["vp"][:, :].rearrange("p (h d) -> p h d", h=H))
                        items.append(mm2)
                        items.append(ev)
                    for ot in (1, 5, 2, 6, 3, 7):
                        f = 0 if ot < 4 else 1
                        dt_ = ot % 4
                        box = {}
                        for ct in range(CT):
                            for ncx in range(NC_CH):
                                def mm(box=box, ct=ct, ncx=ncx, ot=ot):
                                    qp = av_tile(box, ncx, f"fqp{ot}_{ncx}")
                                    nc.tensor.matmul(
                                        qp[:, :],
                                        lhsT=wqkv[:, ct, ot * 128:(ot + 1) * 128],
                                        rhs=xT[:, ct, ncx * 512:(ncx + 1) * 512],
                                        start=(ct == 0), stop=False)
                                items.append(mm)
                        for ncx in range(NC_CH):
                            def mm2(box=box, ncx=ncx, f=f, dt_=dt_):
                                nc.tensor.matmul(
                                    box[ncx][:, :], lhsT=cpvw(dt_),
                                    rhs=tt[0:R, f, ncx * 512:(ncx + 1) * 512],
                                    start=False, stop=True)
                            def ev(box=box, ncx=ncx, ot=ot, dt_=dt_):
                                nc.vector.tensor_scalar_add(
                                    qk[:, ot, ncx * 512:(ncx + 1) * 512],
                                    box[ncx][:, :], cpvb(dt_))
                            items.append(mm2)
                            items.append(ev)
                    return items

                prev_filler = None
                for pair in range(4):
                    st_h = [stpool.tile([128, NT, N], BF, tag="st", name=f"st_{pair}_{i}")
                            for i in range(2)]
                    slots = [(hl, mt) for hl in range(2) for mt in range(NT)]
                    if pair == 0:
                        prev_filler = make_p1_filler()
                    per_slot = 0 if __import__('os').environ.get('NOIL') == '1' else (len(prev_filler) + len(slots) - 1) // len(slots)
                    fi = 0
                    for hl, mt in slots:
                        pb = hl * 64
                        pe = pb + 64
                        msl = slice(mt * 128, (mt + 1) * 128)
                        lp = ps_log.tile([128, N], F32, tag="log")
                        for ncx in range(NC_CH):
                            nsl = slice(ncx * 512, (ncx + 1) * 512)
                            nc.tensor.matmul(lp[:, nsl], lhsT=qk[pb:pe, 4 + pair, msl],
                                             rhs=qk[pb:pe, pair, nsl],
                                             start=True, stop=True,
                                             tile_position=(pb, 0))
                        nc.scalar.activation(st_h[hl][:, mt, :], lp[:, :], AF.Exp,
                                             bias=maskb(mt), scale=SCALE)
                        for _ in range(per_slot):
                            if fi < len(prev_filler):
                                prev_filler[fi]()
                                fi += 1
                    while fi < len(prev_filler):
                        prev_filler[fi]()
                        fi += 1
                    prev_filler = make_av_filler(pair, st_h)

                # ---- tail: attn@v(pair 3) interleaved with u2/t2/proj ----
                p3 = prev_filler   # 36 items: hl0 MMs(16), dn(2), hl1 MMs(16), dn(2)
                u2ps = {}

                def u2_mm(kt, ncx):
                    if "t" not in u2ps:
                        u2ps["t"] = ps_log.tile([128, N], F32, tag="log", name="u2p")
                    nc.tensor.matmul(u2ps["t"][0:R, ncx * 512:(ncx + 1) * 512],
                                     lhsT=cpuw[:, kt, :],
                                     rhs=outT[:, kt, ncx * 512:(ncx + 1) * 512],
                                     start=(kt == 0), stop=(kt == CT - 1))

                def u2_fin(ncx):
                    nsl = slice(ncx * 512, (ncx + 1) * 512)
                    nc.vector.tensor_scalar_add(uu[0:R, 1, nsl],
                                                u2ps["t"][0:R, nsl], cpub())

                # hl0 of pair3 + u2 partial sums over completed pairs
                for item in p3[:18]:
                    item()
                for ncx in range(NC_CH):
                    for kt in range(3):
                        u2_mm(kt, ncx)
                for item in p3[18:]:
                    item()
                t2 = work.tile([R, N], BF)
                for ncx in range(NC_CH):
                    u2_mm(3, ncx)
                    u2_fin(ncx)
                for ncx in range(NC_CH):
                    nsl = slice(ncx * 512, (ncx + 1) * 512)
                    tp2 = ps_av.tile([128, 512], F32, tag="av", name=f"t2p{ncx}")
                    nc.tensor.matmul(tp2[0:R, :], lhsT=F_sb(3), rhs=uu[0:R, 1, nsl],
                                     start=True, stop=True)
                    nc.vector.tensor_copy(t2[0:R, nsl], tp2[0:R, :])

                for ot in range(CT):
                    ppt = ps_log.tile([128, N], F32, tag="log", name=f"ppt{ot}")
                    for kt in range(CT):
                        for ncx in range(NC_CH):
                            nc.tensor.matmul(ppt[:, ncx * 512:(ncx + 1) * 512],
                                             lhsT=wproj[:, kt, ot * 128:(ot + 1) * 128],
                                             rhs=outT[:, kt, ncx * 512:(ncx + 1) * 512],
                                             start=(kt == 0), stop=False)
                    for ncx in range(NC_CH):
                        nsl = slice(ncx * 512, (ncx + 1) * 512)
                        nc.tensor.matmul(ppt[:, nsl], lhsT=cpvw(ot), rhs=t2[0:R, nsl],
                                         start=False, stop=True)
                        po = popool.tile([128, 512], F32, tag="po")
                        nc.vector.tensor_scalar_add(po[:, :], ppt[:, nsl], pbias(ot))
                        eng = nc.sync if (ot + ncx) % 2 == 0 else nc.gpsimd
                        eng.dma_start(out_e[ot * 128:(ot + 1) * 128, nsl], po[:, :])

    nc.compile()
    return nc


def _prep(inputs):
    x = np.asarray(inputs["x"])
    mask = np.asarray(inputs["mask"])
    qkv_w = np.asarray(inputs["qkv_w"], np.float32)
    CP_U_w = np.asarray(inputs["CP_U_w"], np.float32)
    CP_U_b = np.asarray(inputs["CP_U_b"], np.float32)
    CP_V_w = np.asarray(inputs["CP_V_w"], np.float32)
    CP_V_b = np.asarray(inputs["CP_V_b"], np.float32)
    CP_C = np.asarray(inputs["CP_C"], np.float32)
    CP_att = np.asarray(inputs["CP_attention"], np.float32)
    proj_w = np.asarray(inputs["proj_w"], np.float32)
    proj_b = np.asarray(inputs["proj_b"], np.float32)

    fcon = np.zeros((128, 32), np.float32)
    fcon[:, 0:4] = CP_V_b.reshape(CT, 128).T
    fcon[:, 4:8] = (proj_b + CP_V_b).reshape(CT, 128).T
    fcon[0:R, 16] = CP_U_b

    com = {
        "wqkvT": np.ascontiguousarray(qkv_w.T).astype(bf16),
        "cpuwT": np.ascontiguousarray(CP_U_w.T).astype(bf16),
        "cpvwT": np.ascontiguousarray(CP_V_w.T).astype(bf16),
        "cpvw65": np.ascontiguousarray(
            np.concatenate([CP_V_w.T, CP_V_b[None]], 0)).astype(bf16),
        "cpcrt": np.ascontiguousarray(
            np.transpose(CP_C, (2, 0, 1)).reshape(R, R * R)).astype(bf16),
        "cpatt": np.ascontiguousarray(CP_att).astype(bf16),
        "wprojT": np.ascontiguousarray(proj_w.T).astype(bf16),
    }
    in_maps = []
    for b in range(B):
        m = dict(com)
        m["xT"] = np.ascontiguousarray(x[b].T).astype(bf16)
        fc = fcon.copy()
        mb = np.where(mask[b], 0.0, -1e30).astype(np.float32)
        fc[:, 8:16] = mb.reshape(NT, 128).T
        m["fcon"] = fc
        in_maps.append(m)
    return in_maps


LAST_EXEC_NS = None


def kernel(**inputs):
    global LAST_EXEC_NS
    if "nc" not in _CACHE:
        _CACHE["nc"] = _build()
    nc = _CACHE["nc"]
    in_maps = _prep(inputs)
    res = run_bass_kernel_spmd(nc, in_maps, core_ids=list(range(NCORES)))
    LAST_EXEC_NS = res.exec_time_ns
    out = np.stack([np.ascontiguousarray(res.results[i]["out"].T)
                    for i in range(NCORES)])
    return out.astype(np.float32)
